# revision 72
# baseline (speedup 1.0000x reference)
"""Trainium2 Bass kernel for nn_LinformerProjectionEntireOutImg.

Math: the reference's softmax is over a constant tensor -> uniform 1/64, so
the whole net collapses to a linear pipeline:
  T[b,q,i,j]  = sum_p cp[b, p*128+q, i, :] @ wc[p*128+q, :, j]   (q = n mod 128)
  P2[b,e]     = sum_{q,i,j} T[b,q,i,j] * Ered[(q,i,j), e]
  out[b,o,i,j]= sum_m (P2[b, i*8+m] + rel[i*8+m]) * w_next[o, m, j]
where Ered folds the uniform pooling: Ered[nh,s,e] = sum_k E[nh,s,e+64k]/64.

Sharding: core c owns capsule groups q in [16c, 16c+16) (== heads 4c..4c+4),
batch unsharded. Each core reads a disjoint 1/8 of current_pose/w_current and
its 4 heads of the folded E. The pipeline is linear, so each core emits its
partial P2 (32x64 f32, 8 KB) and the unshard is a sum; the tiny affine
stage-3 epilogue (rel add + 8x8 w_next matmul, ~2 MFLOP) runs on host.

Device stage 1: q's are processed in groups g of 4 (PE matmul output base
partition must be a multiple of 32). Per (g, pc8-step) one matmul
  o_ps[32g + u*8+j, (i,b)] += Wblk[(u',p4,m), (u,j)].T @ A[(u',p4,m), (i,b)]
with Wblk block-diagonal over u==u'. Only the compact 128 KB W ships over
DMA (concatenated with the folded E as one 2-KB-row transfer — DMA packets
cap at the SBUF row length, so narrow rows mean small packets); the
block-diagonal tile is built on-chip (memset + 4 strided diagonal-block
copies on vector/gpsimd). 16-step accumulation chains per group, two groups
interleaved so consecutive matmuls hit different PSUM rows. A streams
round-robin across the two HWDGE queues (sync/scalar): one 512 KB leading
chunk per g (4 KB packets) then 256 KB chunks so the arrival cadence stays
finer than PE consumption — a stalled PE resets its p-state ramp and runs
at half clock. Stage-2 (contracting (q,j) x i against the folded E, two
64-partition halves) runs after all chains in PE order so its PSUM->SBUF
casts never stall the PE mid-stream. All device compute is bf16 with f32
PSUM accumulation.
"""

import os

import numpy as np

_STATE: dict = {}

B, OUT_N, POSE = 32, 64, 64
NCORES = 8
NQ = 16  # capsule groups per core; 4 PE groups of 4

def _build_nc():
    import concourse.mybir as mybir
    from concourse import bacc
    from concourse.tile import TileContext

    f32 = mybir.dt.float32
    bf16 = mybir.dt.bfloat16
    nc = bacc.Bacc()
    # A chunks: DMA packets cap at the row length, so the leading chunk per
    # g spans two qt's ([128, 2048] -> 4 KB rows/packets). The rest of each
    # g splits as steps s8-14 ([128, 1792]) plus a tiny s15 piece
    # ([128, 256]) so only the last two chain matmuls are gated on the
    # final bytes + completion-semaphore latency.
    # a_big k = g: qt01; a_smA k = g: s8-14; a_smB k = g: s15
    AB = nc.dram_tensor("a_big", [4, 128, 2048], bf16, kind="ExternalInput")
    ASA = nc.dram_tensor("a_smA", [4, 128, 1792], bf16, kind="ExternalInput")
    ASB = nc.dram_tensor("a_smB", [4, 128, 256], bf16, kind="ExternalInput")
    # we_pack: compact W [128=(u,p4,m), 512=(g, pc8, j)] (expanded on-chip
    # so no zeros ride the DMA) concatenated with the pool-folded E
    # [128=(q,j), 512=(i,e)] — one 2-KB-row transfer instead of two
    # 1-KB-row transfers front-loading both queues with small packets
    WE = nc.dram_tensor("we_pack", [128, 1024], bf16, kind="ExternalInput")
    OUT = nc.dram_tensor("out", [32, 64], f32, kind="ExternalOutput")

    with TileContext(nc) as tc:
        with (
            tc.tile_pool(name="apool", bufs=NQ) as apool,
            tc.tile_pool(name="cpool", bufs=1) as cpool,
            tc.tile_pool(name="spool", bufs=1) as spool,
            tc.tile_pool(name="pp1", bufs=1, space="PSUM") as pp1,
            tc.tile_pool(name="pp2", bufs=1, space="PSUM") as pp2,
        ):
            # Compact W + folded E ride as one 256 KB transfer first on
            # sync; scalar starts its A stream immediately. The
            # block-diagonal weight tile is built on-chip: memset zeros +
            # one strided diagonal-block copy per u on vector/gpsimd.
            wes = cpool.tile([128, 1024], bf16, tag="we")
            nc.sync.dma_start(out=wes[:], in_=WE[:])
            wt = cpool.tile([128, 2048], bf16, tag="w")
            nc.vector.memset(wt[:, 0:1024], 0)
            nc.gpsimd.memset(wt[:, 1024:2048], 0)
            wt_r = wt[:].rearrange("p (g s u j) -> p g s u j", g=4, s=16, u=4, j=8)
            for u in range(4):
                eng = (nc.vector, nc.gpsimd)[u % 2]
                eng.tensor_copy(
                    wt_r[u * 32 : (u + 1) * 32, :, :, u, :],
                    wes[u * 32 : (u + 1) * 32, 0:512].rearrange(
                        "p (g s j) -> p g s j", g=4, s=16, j=8
                    ),
                )
            # sync streams g0/g2 (+W, +out), scalar streams g1/g3 (+E),
            # interleaved so arrival matches PE consumption order
            # per-queue entry order (sync = g0/g2 stream, scalar = g1/g3):
            #   big qt01, smA s8-14, smB s15, big qt01(next g), smA, smB
            big = [None] * 4
            smA = [None] * 4
            smB = [None] * 4
            for gp in range(2):
                for u in range(2):
                    g = gp * 2 + u
                    bt = apool.tile([128, 2048], bf16, tag="ab")
                    (nc.sync, nc.scalar)[u].dma_start(out=bt[:], in_=AB[g])
                    big[g] = bt
                for half in range(2):
                    for u in range(2):
                        g = gp * 2 + u
                        if half == 0:
                            st = apool.tile([128, 1792], bf16, tag="asa")
                            (nc.sync, nc.scalar)[u].dma_start(
                                out=st[:], in_=ASA[g]
                            )
                            smA[g] = st
                        else:
                            st = apool.tile([128, 256], bf16, tag="asb")
                            (nc.sync, nc.scalar)[u].dma_start(
                                out=st[:], in_=ASB[g]
                            )
                            smB[g] = st

            def a_slice(g, s):
                if s < 8:
                    return big[g][:, s * 256 : (s + 1) * 256]
                if s < 15:
                    return smA[g][:, (s - 8) * 256 : (s - 7) * 256]
                return smB[g][:]

            # PE out base partitions are limited to {0,32,64}; use one
            # [64,256] PSUM tile per g-pair so each group lands at base 0/32.
            o_psA = pp1.tile([64, 256], f32, tag="o_psA")
            o_psB = pp1.tile([64, 256], f32, tag="o_psB")
            o_ps = (o_psA, o_psB)
            o_sb = spool.tile([128, 256], bf16, tag="o_sb")
            p2 = pp2.tile([32, 64], f32, tag="p2")

            def stage2_half(h):
                # half 1 is on the critical tail: use the scalar engine,
                # which is idle once its DMA issues are done
                if h == 0:
                    nc.vector.tensor_copy(o_sb[0:64, :], o_ps[0][:])
                else:
                    nc.scalar.copy(o_sb[64:128, :], o_ps[1][:])
                for i in range(8):
                    nc.tensor.matmul(
                        p2[:],
                        o_sb[h * 64 : (h + 1) * 64, i * 32 : (i + 1) * 32],
                        wes[h * 64 : (h + 1) * 64, 512 + i * 64 : 576 + i * 64],
                        start=(h == 0 and i == 0),
                        stop=(h == 1 and i == 7),
                    )

            # stage-2 halves run AFTER all chains in PE order: half 0's
            # PSUM->SBUF cast then overlaps the pair-1 chains instead of
            # stalling the PE (a stall resets the p-state ramp)
            for gp in range(2):
                for s in range(16):  # pc8 step
                    for u in range(2):
                        g = gp * 2 + u
                        nc.tensor.matmul(
                            o_ps[gp][u * 32 : (u + 1) * 32, :],
                            wt[:, g * 512 + s * 32 : g * 512 + s * 32 + 32],
                            a_slice(g, s),
                            start=(s == 0),
                            stop=(s == 15),
                        )
            stage2_half(0)
            stage2_half(1)

            v_sb = spool.tile([32, 64], f32, tag="v")
            nc.scalar.copy(v_sb[:], p2[:])
            nc.scalar.dma_start(out=OUT[:], in_=v_sb[:])
    nc.finalize()
    return nc


def _prepack(current_pose, w_current, E_proj):
    import ml_dtypes

    bf16 = ml_dtypes.bfloat16
    # A[c, g*4+qt, (u',p4,m), (pc8l,i,b)]
    #   = cp[b, ((qt*4+pc8l)*4+p4)*128 + 16c + 4g + u', i*8+m]
    cp = np.ascontiguousarray(current_pose, dtype=np.float32)
    a9 = cp.reshape(B, 4, 4, 4, 8, 4, 4, 8, 8)  # (b,qt,pc8l,p4,c,g,u',i,m)
    a_all = np.ascontiguousarray(
        a9.transpose(4, 5, 1, 6, 3, 8, 2, 7, 0), dtype=bf16
    ).reshape(NCORES, NQ, 128, 1024)
    # big chunks pair qt's along the row: (g, h) -> cols (qt%2, pc8l, i, b)
    a_pair = (
        a_all.reshape(NCORES, 8, 2, 128, 1024)
        .transpose(0, 1, 3, 2, 4)
        .reshape(NCORES, 8, 128, 2048)
    )
    a_big = np.ascontiguousarray(a_pair[:, [0, 2, 4, 6]])
    # s8-14 = all of qt2 plus the first 768 cols of qt3; s15 = qt3[768:]
    qt2 = a_all[:, [2, 6, 10, 14]]
    qt3 = a_all[:, [3, 7, 11, 15]]
    a_smA = np.ascontiguousarray(np.concatenate([qt2, qt3[..., 0:768]], axis=3))
    a_smB = np.ascontiguousarray(qt3[..., 768:1024])
    # compact W[c, (u,p4,m), (g,pc8,j)] = wc[(pc8*4+p4)*128 + 16c+4g+u, m, j]
    wc = np.asarray(w_current, dtype=np.float32).reshape(16, 4, 8, 4, 4, 8, 8)
    # (pc8, p4, c, g, u, m, j) -> (c, u, p4, m, g, pc8, j)
    w_all = np.ascontiguousarray(
        wc.transpose(2, 4, 1, 5, 3, 0, 6), dtype=bf16
    ).reshape(NCORES, 128, 512)
    # E[c, (q,j), (i,e)] from the pool-folded projection (1/64 baked in)
    er = np.asarray(E_proj, dtype=np.float32).reshape(32, 256, 4, 64).sum(axis=2)
    er = (er / 64.0).reshape(8, 4, 4, 8, 8, 64)  # (c, nh_loc, s_hi, i, j, e)
    e_all = np.ascontiguousarray(
        er.transpose(0, 1, 2, 4, 3, 5), dtype=bf16
    ).reshape(NCORES, 128, 512)
    we_all = np.concatenate([w_all, e_all], axis=2)
    return [
        {
            "a_big": a_big[c],
            "a_smA": a_smA[c],
            "a_smB": a_smB[c],
            "we_pack": we_all[c],
        }
        for c in range(NCORES)
    ]


def kernel(current_pose, w_current, w_next, E_proj, rel_embedd):
    from concourse import bass_utils

    if "nc" not in _STATE:
        _STATE["nc"] = _build_nc()
    nc = _STATE["nc"]
    in_maps = _prepack(current_pose, w_current, E_proj)
    trace = os.environ.get("KERNEL_TRACE") == "1"
    res = bass_utils.run_bass_kernel_spmd(
        nc, in_maps, core_ids=list(range(NCORES)), trace=trace
    )
    _STATE["last_result"] = res
    v = np.zeros((B, POSE), dtype=np.float32)
    for c in range(NCORES):
        v += res.results[c]["out"]
    # host epilogue (~2 MFLOP): rel add + next-layer 8x8 pose matmul
    npc = v + np.asarray(rel_embedd, dtype=np.float32).reshape(1, POSE)
    wn = np.asarray(w_next, dtype=np.float32)  # (OUT_N, 8, 8)
    out = np.einsum("bim,omj->boij", npc.reshape(B, 8, 8), wn, optimize=True)
    return np.ascontiguousarray(out.reshape(B, 1, OUT_N, POSE), dtype=np.float32)
